# revision 1
# baseline (speedup 1.0000x reference)
"""Chamfer loss (nn_ChamLoss) Trainium2 kernel.

Data-parallel over batch: each of the 8 cores handles 8 samples.
Per core:
  1. decode: global-avg-pool latents -> augmented matmul against the
     combined [Ldelta;Rdelta;Lmu;Rmu] weight (eye selection folded in via
     s/(1-s) scaling of the rhs) -> coordinate planes [8, 1448].
  2. build augmented cdist operands LF=[x,y,z,1,sq], RF=[-2x,-2y,-2z,sq,1]
     (5 rows per sample, partition-interleaved).
  3. for both directions, per sample: K=5 matmuls emit squared-distance
     tiles into PSUM; DVE min-reduce; clamp+sqrt; sum -> scalar partial.
Host sums the 8 partials and scales by 1/(B*N).
"""

import numpy as np
from contextlib import ExitStack

import concourse.bass as bass
import concourse.bacc as bacc
import concourse.tile as tile
from concourse import mybir

F32 = mybir.dt.float32
B = 64
C = 256
HW = 64            # 8*8 spatial, average-pooled
N = 1448           # vertices
J3 = 3 * N         # 4344
KAUG = 2 * C + 2   # 514 augmented contraction dim for decode
NCORES = 8
BL = B // NCORES   # 8 local samples per core
CHUNKS = [(0, 512), (512, 512), (1024, 424)]   # m-chunks of 1448
NT = 12            # n-tiles of 128 (last has 40 rows)
LAST_M = N - 11 * 128  # 40

_CACHE = {}


def _build_program():
    nc = bass.Bass()
    x_d = nc.dram_tensor("x", [2, 128, BL * HW], F32, kind="ExternalInput")
    t_d = nc.dram_tensor("t", [2, 128, BL * HW], F32, kind="ExternalInput")
    w_d = nc.dram_tensor("w", [KAUG, J3], F32, kind="ExternalInput")
    sb_d = nc.dram_tensor("sb", [128, 2 * BL], F32, kind="ExternalInput")
    smu_d = nc.dram_tensor("smu", [2, BL], F32, kind="ExternalInput")
    o_d = nc.dram_tensor("o", [1, 1], F32, kind="ExternalOutput")

    AF = mybir.ActivationFunctionType
    ALU = mybir.AluOpType

    with tile.TileContext(nc) as tc, ExitStack() as ctx:
        perm = ctx.enter_context(tc.tile_pool(name="perm", bufs=1))
        planes = ctx.enter_context(tc.tile_pool(name="planes", bufs=1))
        planes2 = ctx.enter_context(tc.tile_pool(name="planes2", bufs=2))
        rpool = ctx.enter_context(tc.tile_pool(name="rpool", bufs=2))

        # ---- resident weights ----
        wt = []
        for kk in range(5):
            kw = 128 if kk < 4 else KAUG - 512
            wtile = perm.tile([kw, J3], F32, tag=f"w{kk}")
            nc.sync.dma_start(wtile[:, :], w_d[kk * 128:kk * 128 + kw, :])
            wt.append(wtile)

        sbt = perm.tile([128, 2 * BL], F32, tag="sbt")
        nc.sync.dma_start(sbt[:, :], sb_d[:, :])

        # ---- latents -> latA (augmented rhs for decode) ----
        smu_sb = perm.tile([2, BL], F32, tag="smu_sb")
        nc.sync.dma_start(smu_sb[:, :], smu_d[:, :])
        latA = {}
        for name, src in (("x", x_d), ("t", t_d)):
            la = perm.tile([128, 32], F32, tag=f"latA_{name}")
            lat = perm.tile([128, 2 * BL], F32, tag=f"lat_{name}")
            for h in range(2):
                xt = planes2.tile([128, BL * HW], F32, tag="xraw")
                nc.sync.dma_start(xt[:, :], src[h, :, :])
                nc.vector.tensor_reduce(
                    out=lat[:, h * BL:(h + 1) * BL],
                    in_=xt[:, :].rearrange("p (b w) -> p b w", w=HW),
                    op=ALU.add, axis=mybir.AxisListType.X,
                )
            # blocks: [lat0*(1-s), lat1*(1-s), lat0*s, lat1*s]
            nc.vector.tensor_mul(la[:, 0:8], lat[:, 0:8], sbt[:, 0:8])
            nc.vector.tensor_mul(la[:, 8:16], lat[:, 8:16], sbt[:, 0:8])
            nc.vector.tensor_mul(la[:, 16:24], lat[:, 0:8], sbt[:, 8:16])
            nc.vector.tensor_mul(la[:, 24:32], lat[:, 8:16], sbt[:, 8:16])
            latA[name] = la

        # ---- decode -> persistent coordinate planes [x, y, z, sq] per tensor
        PL = {}
        with tc.tile_pool(name="psdec", bufs=2, space="PSUM") as psdec:
            for name in ("x", "t"):
                la = latA[name]
                pl = []
                for q in range(3):
                    pq = psdec.tile([BL, N], F32, tag="pdec")
                    for (m0, mw) in CHUNKS:
                        for kk in range(5):
                            lhsT = la[:, kk * 8:kk * 8 + 8] if kk < 4 \
                                else smu_sb[:, :]
                            nc.tensor.matmul(
                                pq[:, m0:m0 + mw],
                                lhsT=lhsT,
                                rhs=wt[kk][:, q * N + m0:q * N + m0 + mw],
                                start=(kk == 0), stop=(kk == 4),
                            )
                    p_q = perm.tile([BL, N], F32, tag=f"p{name}{q}")
                    nc.scalar.copy(p_q[:, :], pq[:, :])
                    pl.append(p_q)
                sq = perm.tile([BL, N], F32, tag=f"sq{name}")
                tmp = planes.tile([BL, N], F32, tag="sqtmp")
                nc.vector.tensor_mul(sq[:, :], pl[0][:, :], pl[0][:, :])
                nc.vector.tensor_mul(tmp[:, :], pl[1][:, :], pl[1][:, :])
                nc.vector.tensor_add(sq[:, :], sq[:, :], tmp[:, :])
                nc.vector.tensor_mul(tmp[:, :], pl[2][:, :], pl[2][:, :])
                nc.vector.tensor_add(sq[:, :], sq[:, :], tmp[:, :])
                PL[name] = pl + [sq]

        # ---- chamfer distance passes ----
        acc = perm.tile([128, 16], F32, tag="acc")
        ones = perm.tile([128, 1], F32, tag="ones")
        nc.vector.memset(ones[:, :], 1.0)
        onesrow = perm.tile([1, N], F32, tag="onesrow")
        nc.vector.memset(onesrow[:, :], 1.0)

        stage = ctx.enter_context(tc.tile_pool(name="stage", bufs=3))
        with tc.tile_pool(name="psc", bufs=4, space="PSUM") as psc:
            for d in range(2):
                apl = PL["x"] if d == 0 else PL["t"]   # lhsT-side planes
                bpl = PL["t"] if d == 0 else PL["x"]   # rhs-side planes
                for b in range(BL):
                    # per-sample staged operands at base partition 0:
                    # lt = [-2x, -2y, -2z, 1, sq] of A; rt = [x, y, z, sq, 1]
                    # of B; d2 = lt.T @ rt
                    lt = stage.tile([5, N], F32, tag="lt")
                    rt = stage.tile([5, N], F32, tag="rt")
                    for q in range(3):
                        nc.sync.dma_start(lt[q:q + 1, :], apl[q][b:b + 1, :])
                        nc.sync.dma_start(rt[q:q + 1, :], bpl[q][b:b + 1, :])
                    nc.sync.dma_start(lt[4:5, :], apl[3][b:b + 1, :])
                    nc.sync.dma_start(rt[3:4, :], bpl[3][b:b + 1, :])
                    nc.sync.dma_start(lt[3:4, :], onesrow[:, :])
                    nc.sync.dma_start(rt[4:5, :], onesrow[:, :])
                    nc.vector.tensor_scalar_mul(lt[0:3, :], lt[0:3, :], -2.0)
                    rmins = rpool.tile([128, 36], F32, tag="rmins")
                    nc.vector.memset(rmins[:, :], 0.0)
                    for nt in range(NT):
                        M = LAST_M if nt == NT - 1 else 128
                        n0 = nt * 128
                        for j, (m0, mw) in enumerate(CHUNKS):
                            ps = psc.tile([128, 512], F32, tag="ps")
                            nc.tensor.matmul(
                                ps[:M, :mw],
                                lhsT=lt[:, n0:n0 + M],
                                rhs=rt[:, m0:m0 + mw],
                            )
                            nc.vector.tensor_reduce(
                                out=rmins[:M, nt * 3 + j:nt * 3 + j + 1],
                                in_=ps[:M, :mw],
                                op=ALU.min, axis=mybir.AxisListType.X,
                            )
                    t12 = rpool.tile([128, 12], F32, tag="t12")
                    nc.vector.tensor_reduce(
                        out=t12[:, :],
                        in_=rmins[:, :].rearrange("p (a c) -> p a c", c=3),
                        op=ALU.min, axis=mybir.AxisListType.X,
                    )
                    nc.vector.tensor_scalar_max(t12[:, :], t12[:, :], 1e-12)
                    nc.scalar.activation(t12[:, :], t12[:, :], AF.Sqrt)
                    nc.vector.tensor_reduce(
                        out=acc[:, d * 8 + b:d * 8 + b + 1],
                        in_=t12[:, :],
                        op=ALU.add, axis=mybir.AxisListType.X,
                    )

            # ---- final partition sum -> scalar ----
            srow = perm.tile([128, 1], F32, tag="srow")
            nc.vector.tensor_reduce(
                out=srow[:, :], in_=acc[:, :],
                op=ALU.add, axis=mybir.AxisListType.X,
            )
            with tc.tile_pool(name="psfin", bufs=1, space="PSUM") as psfin:
                pfin = psfin.tile([1, 1], F32, tag="pfin")
                nc.tensor.matmul(pfin[:, :], lhsT=srow[:, 0:1],
                                 rhs=ones[:, 0:1])
                osb = perm.tile([1, 1], F32, tag="osb")
                nc.scalar.copy(osb[:, :], pfin[:, :])
                nc.sync.dma_start(o_d[:, :], osb[:, :])

    return nc


F16 = mybir.dt.float16
NPAD = 1536          # N padded to 12*128 for full-M matmuls / transposes
ACT_COPY = 12        # of 12 PSUM->SBUF fp16 copies per sample, how many on ACT
GPS_COL = 0          # colmin TT column split point: [0,GPS_COL) on gpsimd


def _build_program_v2():
    nc = bacc.Bacc(None, target_bir_lowering=False)
    x_d = nc.dram_tensor("x", [2, 128, BL * HW], F32, kind="ExternalInput")
    t_d = nc.dram_tensor("t", [2, 128, BL * HW], F32, kind="ExternalInput")
    w_d = nc.dram_tensor("w", [KAUG, J3], F16, kind="ExternalInput")
    sb_d = nc.dram_tensor("sb", [128, 2 * BL], F32, kind="ExternalInput")
    smu_d = nc.dram_tensor("smu", [2, BL], F16, kind="ExternalInput")
    o_d = nc.dram_tensor("o", [1, 1], F32, kind="ExternalOutput")
    # internal DRAM bounce for assembled planes (fp16), 7 blocks of width
    # NPAD: [x, y, z, 1, 1, sqhi, sqlo]; cols [N:NPAD] of each block zero
    pd_d = {name: nc.dram_tensor(f"pd_{name}", [BL, 7 * NPAD], F16)
            for name in ("x", "t")}

    AF = mybir.ActivationFunctionType
    ALU = mybir.AluOpType
    from concourse.masks import make_identity

    with tile.TileContext(nc) as tc, ExitStack() as ctx:
        perm = ctx.enter_context(tc.tile_pool(name="perm", bufs=1))

        sbt = perm.tile([128, 2 * BL], F32, tag="sbt")
        nc.sync.dma_start(sbt[:, :], sb_d[:, :])
        smu_sb = perm.tile([2, BL], F16, tag="smu_sb")
        nc.sync.dma_start(smu_sb[:, :], smu_d[:, :])
        ident = perm.tile([128, 128], F16, tag="ident")
        make_identity(nc, ident[:, :])

        # ---- decode phase ----
        with tc.tile_pool(name="wpool", bufs=1) as wpool, \
             tc.tile_pool(name="dtmp", bufs=2) as dtmp, \
             tc.tile_pool(name="plp", bufs=2) as plp, \
             tc.tile_pool(name="psdec", bufs=2, space="PSUM") as psdec:
            wt = []
            for kk in range(5):
                kw = 128 if kk < 4 else KAUG - 512
                wtile = wpool.tile([kw, J3], F16, tag=f"w{kk}")
                nc.sync.dma_start(wtile[:, :], w_d[kk * 128:kk * 128 + kw, :])
                wt.append(wtile)

            latA = {}
            for name, src in (("x", x_d), ("t", t_d)):
                la = perm.tile([128, 32], F16, tag=f"latA_{name}")
                lat = perm.tile([128, 2 * BL], F32, tag=f"lat_{name}")
                for h in range(2):
                    xt = dtmp.tile([128, BL * HW], F32, tag="xraw")
                    nc.sync.dma_start(xt[:, :], src[h, :, :])
                    nc.vector.tensor_reduce(
                        out=lat[:, h * BL:(h + 1) * BL],
                        in_=xt[:, :].rearrange("p (b w) -> p b w", w=HW),
                        op=ALU.add, axis=mybir.AxisListType.X,
                    )
                nc.vector.tensor_mul(la[:, 0:8], lat[:, 0:8], sbt[:, 0:8])
                nc.vector.tensor_mul(la[:, 8:16], lat[:, 8:16], sbt[:, 0:8])
                nc.vector.tensor_mul(la[:, 16:24], lat[:, 0:8], sbt[:, 8:16])
                nc.vector.tensor_mul(la[:, 24:32], lat[:, 8:16], sbt[:, 8:16])
                latA[name] = la

            for name in ("x", "t"):
                la = latA[name]
                pl5 = plp.tile([BL, 7 * NPAD], F16, tag="pl",
                               name=f"pl_{name}")
                # zero the [N:NPAD] pad gap of every block
                for blk in range(7):
                    nc.vector.memset(pl5[:, blk * NPAD + N:(blk + 1) * NPAD],
                                     0.0)
                for q in range(3):
                    pq = psdec.tile([BL, N], F32, tag="pdec")
                    for (m0, mw) in CHUNKS:
                        for kk in range(5):
                            lhsT = la[:, kk * 8:kk * 8 + 8] if kk < 4 \
                                else smu_sb[:, :]
                            nc.tensor.matmul(
                                pq[:, m0:m0 + mw],
                                lhsT=lhsT,
                                rhs=wt[kk][:, q * N + m0:q * N + m0 + mw],
                                start=(kk == 0), stop=(kk == 4),
                            )
                    nc.scalar.copy(pl5[:, q * NPAD:q * NPAD + N], pq[:, :])
                nc.vector.memset(pl5[:, 3 * NPAD:3 * NPAD + N], 1.0)
                nc.vector.memset(pl5[:, 4 * NPAD:4 * NPAD + N], 1.0)
                # sq from the fp16-rounded coords, split into fp16 hi + lo
                t1 = dtmp.tile([BL, N], F32, tag="sq1")
                t2 = dtmp.tile([BL, N], F32, tag="sq2")
                xb = pl5[:, 0:N]
                yb = pl5[:, NPAD:NPAD + N]
                zb = pl5[:, 2 * NPAD:2 * NPAD + N]
                nc.vector.tensor_mul(t1[:, :], xb, xb)
                nc.vector.tensor_mul(t2[:, :], yb, yb)
                nc.vector.tensor_add(t1[:, :], t1[:, :], t2[:, :])
                nc.vector.tensor_mul(t2[:, :], zb, zb)
                nc.vector.tensor_add(t1[:, :], t1[:, :], t2[:, :])
                hib = pl5[:, 5 * NPAD:5 * NPAD + N]
                nc.vector.tensor_copy(hib, t1[:, :])
                nc.vector.tensor_copy(t2[:, :], hib)
                nc.vector.tensor_sub(t1[:, :], t1[:, :], t2[:, :])
                nc.vector.tensor_copy(pl5[:, 6 * NPAD:6 * NPAD + N], t1[:, :])
                nc.sync.dma_start(pd_d[name][:, :], pl5[:, :])

        acc = perm.tile([128, BL], F32, tag="acc")
        ones = perm.tile([128, 1], F32, tag="ones")
        nc.vector.memset(ones[:, :], 1.0)

        # ---- chamfer phase ----
        stage = ctx.enter_context(tc.tile_pool(name="stage", bufs=4))
        spool = ctx.enter_context(tc.tile_pool(name="spool", bufs=4))
        rpool = ctx.enter_context(tc.tile_pool(name="rpool", bufs=2))
        with tc.tile_pool(name="psc", bufs=2, space="PSUM") as psc, \
             tc.tile_pool(name="pst", bufs=1, space="PSUM") as pst:
            apd, bpd = pd_d["x"], pd_d["t"]
            if True:
                for b in range(BL):
                    # lt = [-2x,-2y,-2z,1,1,sqhi,sqlo]  (n-side, padded)
                    # rt = [x,y,z,sqhi,sqlo,1,1]        (m-side)
                    lt = stage.tile([7, NPAD], F16, tag="lt")
                    rt = stage.tile([7, N], F16, tag="rt")
                    nc.sync.dma_start(
                        lt[:, :],
                        apd[b:b + 1, :].rearrange(
                            "p (q v) -> (p q) v", v=NPAD))
                    nc.sync.dma_start(
                        rt[0:3, :],
                        bpd[b:b + 1, 0:3 * NPAD].rearrange(
                            "p (q v) -> (p q) v", v=NPAD)[:, 0:N])
                    nc.sync.dma_start(
                        rt[3:5, :],
                        bpd[b:b + 1, 5 * NPAD:7 * NPAD].rearrange(
                            "p (q v) -> (p q) v", v=NPAD)[:, 0:N])
                    nc.sync.dma_start(
                        rt[5:7, :],
                        bpd[b:b + 1, 3 * NPAD:5 * NPAD].rearrange(
                            "p (q v) -> (p q) v", v=NPAD)[:, 0:N])
                    nc.vector.tensor_scalar_mul(lt[0:3, :], lt[0:3, :], -2.0)

                    R = rpool.tile([128, NPAD], F16, tag="R")
                    nc.vector.memset(R[:, N:NPAD], 0.0)
                    rball = rpool.tile([128, 12, 181], F16, tag="rball")
                    for nt in range(NT):
                        n0 = nt * 128
                        ps = psc.tile([128, N], F32, tag="ps")
                        for (m0, mw) in CHUNKS:
                            nc.tensor.matmul(
                                ps[:, m0:m0 + mw],
                                lhsT=lt[:, n0:n0 + 128],
                                rhs=rt[:, m0:m0 + mw],
                            )
                        if nt == 0:
                            S = R
                        else:
                            S = spool.tile([128, N], F16, tag="S")
                        if nt < ACT_COPY:
                            nc.scalar.copy(S[:, :N], ps[:, :])
                        else:
                            nc.vector.tensor_copy(S[:, :N], ps[:, :])
                        nc.vector.tensor_reduce(
                            out=rball[:, nt, :],
                            in_=S[:, :N].rearrange("p (a c) -> p a c", c=8),
                            op=ALU.min, axis=mybir.AxisListType.X,
                        )
                        if nt > 0:
                            M = LAST_M if nt == NT - 1 else 128
                            nc.vector.tensor_tensor(
                                R[:M, :N], R[:M, :N], S[:M, :N],
                                op=ALU.min)

                    cm24 = rpool.tile([128, 24], F16, tag="cm24")
                    nc.vector.tensor_reduce(
                        out=cm24[:, 0:12], in_=rball[:, :, :],
                        op=ALU.min, axis=mybir.AxisListType.X,
                    )
                    pt = pst.tile([128, 12, 128], F16, tag="pt")
                    for ct in range(12):
                        nc.tensor.transpose(
                            pt[:, ct, :], R[:, ct * 128:(ct + 1) * 128],
                            ident[:, :])
                    nc.vector.tensor_reduce(
                        out=cm24[:, 12:24], in_=pt[:, :, :],
                        op=ALU.min, axis=mybir.AxisListType.X,
                    )
                    t24 = rpool.tile([128, 24], F32, tag="t24")
                    nc.vector.tensor_copy(t24[:, :], cm24[:, :])
                    nc.vector.tensor_scalar_max(t24[:, :], t24[:, :], 1e-12)
                    nc.scalar.activation(t24[:, :], t24[:, :], AF.Sqrt)
                    nc.vector.tensor_reduce(
                        out=acc[:, b:b + 1], in_=t24[:, :],
                        op=ALU.add, axis=mybir.AxisListType.X,
                    )

            srow = perm.tile([128, 1], F32, tag="srow")
            nc.vector.tensor_reduce(
                out=srow[:, :], in_=acc[:, :],
                op=ALU.add, axis=mybir.AxisListType.X,
            )
            pfin = pst.tile([1, 1], F32, tag="pt", name="pfin")
            nc.tensor.matmul(pfin[:, :], lhsT=srow[:, 0:1],
                             rhs=ones[:, 0:1])
            osb = perm.tile([1, 1], F32, tag="osb")
            nc.scalar.copy(osb[:, :], pfin[:, :])
            nc.sync.dma_start(o_d[:, :], osb[:, :])

    nc.compile()
    return nc


def _prep_maps(inputs, targets, eye_labels, left_mu, left_delta,
               right_mu, right_delta):
    """Build per-core input maps (host-side sharding + layout prep)."""
    def group(a):
        # [4344(=3v+q), ...] -> column-grouped [3*1448] with col q*N+v
        return np.ascontiguousarray(
            a.reshape(N, 3).T.reshape(J3) if a.ndim == 1
            else a.reshape(N, 3, -1).transpose(1, 0, 2)
        )

    W = np.empty((KAUG, J3), np.float16)
    # delta.T grouped: W[c, q*N+v] = delta[3v+q, c] / 64
    ldt = (left_delta.astype(np.float32) / np.float32(64.0)).T   # [256, 4344]
    rdt = (right_delta.astype(np.float32) / np.float32(64.0)).T
    W[0:C] = ldt.reshape(C, N, 3).transpose(0, 2, 1).reshape(C, J3)
    W[C:2 * C] = rdt.reshape(C, N, 3).transpose(0, 2, 1).reshape(C, J3)
    W[512] = group(left_mu[:, 0].astype(np.float32))
    W[513] = group(right_mu[:, 0].astype(np.float32))

    s = eye_labels.astype(np.float32)
    maps = []
    for k in range(NCORES):
        b0 = k * BL
        xs = np.ascontiguousarray(
            inputs[b0:b0 + BL].reshape(BL, 2, 128, HW)
            .transpose(1, 2, 0, 3).reshape(2, 128, BL * HW)).astype(np.float32)
        ts = np.ascontiguousarray(
            targets[b0:b0 + BL].reshape(BL, 2, 128, HW)
            .transpose(1, 2, 0, 3).reshape(2, 128, BL * HW)).astype(np.float32)
        s8 = s[b0:b0 + BL]
        sb = np.empty((128, 2 * BL), np.float32)
        sb[:, 0:BL] = 1.0 - s8
        sb[:, BL:2 * BL] = s8
        smu = np.stack([1.0 - s8, s8]).astype(np.float16)
        maps.append({"x": xs, "t": ts, "w": W, "sb": sb, "smu": smu})
    return maps


LAST_EXEC_NS = None
LAST_RESULT = None


def kernel(inputs, targets, eye_labels, left_mu, left_delta,
           right_mu, right_delta, trace=False):
    global LAST_EXEC_NS, LAST_RESULT
    from concourse.bass_utils import run_bass_kernel_spmd

    if "nc" not in _CACHE:
        _CACHE["nc"] = _build_program_v2()
    nc = _CACHE["nc"]

    maps = _prep_maps(np.asarray(inputs), np.asarray(targets),
                      np.asarray(eye_labels), np.asarray(left_mu),
                      np.asarray(left_delta), np.asarray(right_mu),
                      np.asarray(right_delta))
    res = run_bass_kernel_spmd(nc, maps, core_ids=list(range(NCORES)),
                               trace=trace)
    LAST_EXEC_NS = res.exec_time_ns
    LAST_RESULT = res
    total = sum(float(r["o"][0, 0]) for r in res.results)
    return np.float32(total / (B * N))

